# revision 4
# baseline (speedup 1.0000x reference)
"""Subject-routed batched matmul for Trainium2 (8 NeuronCores, SPMD data-parallel).

out[b, d, t] = sum_c x[b, c, t] * weights[subjects[b], c, d]

Strategy:
- Data-parallel over batch B=128 across 8 cores (16 batches each).
- Host-side: gather per-batch weights (weights[subjects], tiny), cast x and w
  to fp16. The tolerance gate is rel_err < 2e-2 and fp16-in/fp16-out measures
  ~5e-4, so single fp16 (2 B/elem) halves HBM traffic vs fp32: 34 MiB/core
  total -> ~100 us roofline at the ~358 GB/s per-core HBM limit.
- Device: per batch, out[b] (256d, 2048t) = w[b].T @ x[b], tiled K=2x128
  (contraction over c), M=2x128 (d -> PSUM partitions), N=4x512 (t, one
  PSUM bank per tile). 2 matmuls per PSUM bank, fp16 at 1 cycle/row.
- Engine assignment (v3, informed by v1/v2 traces):
  * sync (SP HWDGE): x loads only — a clean stream of 1 MiB transfers with
    4 KiB/partition descriptors. (v2 interleaved the weight slices into this
    FIFO and lost ~5 us to head-of-line blocking + slower dispatches.)
  * gpsimd (SWDGE): weight loads (separate queue -> SDMA round-robin keeps
    all engines fed) + store dispatches. The bulk weight piece is delayed
    behind batch 1's x arrival so the first ~15 us of HBM bandwidth goes to
    the critical x stream (v1 lost ~3.5 us of PE idle + a HAM clock dip to
    this competition).
  * vector + scalar: PSUM f32 -> SBUF fp16 casts split by n parity (v1 had
    all 128 casts on DVE at ~690 ns each = 80 us busy, which gated the
    drain at ~220 GB/s once loads finished).
  * Last two batches store per n-chunk to shorten the drain tail.
"""

import sys

for _p in ("/opt/trn_rl_repo", "/root/.axon_site/_ro/trn_rl_repo"):
    if _p not in sys.path:
        sys.path.append(_p)

import numpy as np

import concourse.mybir as mybir
import concourse.tile as tile
from concourse import bacc
from concourse.bass_utils import run_bass_kernel_spmd

B, C, D, T, N_SUBJECTS = 128, 256, 256, 2048, 8
N_CORES = 8
BPC = B // N_CORES  # batches per core

KC = C // 128  # k chunks (contraction dim on partitions)
MC = D // 128  # m chunks (output partition dim)
NT = 512       # n tile (one PSUM bank of f32)
NC_ = T // NT  # n chunks

F32 = mybir.dt.float32
F16 = mybir.dt.float16

# weights for batches [0, WSPLIT) load immediately; the rest wait for xt[1]
WSPLIT = 4

_compiled = None


def _build():
    nc = bacc.Bacc("TRN2", target_bir_lowering=False, debug=False)
    # x1[b, c, t] fp16 (plain reshape of the fp32 input, cast on host)
    # wp[p, b, k, d] fp16 — host-pre-packed to the SBUF layout so the weight
    # DMA is contiguous per partition.
    x_d = nc.dram_tensor("x1", [BPC, C, T], F16, kind="ExternalInput")
    w_d = nc.dram_tensor("wp", [128, BPC, KC, D], F16, kind="ExternalInput")
    o_d = nc.dram_tensor("out", [BPC, D, T], F16, kind="ExternalOutput")

    with tile.TileContext(nc) as tc:
        with (
            tc.tile_pool(name="wpool", bufs=1) as wpool,
            tc.tile_pool(name="xpool", bufs=8) as xpool,
            tc.tile_pool(name="opool", bufs=4) as opool,
            tc.tile_pool(name="psum", bufs=8, space="PSUM") as psum,
        ):
            wt = wpool.tile([128, BPC, KC, D], F16)
            # PE warmup: the HAM clock gate boots at 1.2 GHz and needs
            # ~3.4 us of sustained matmul activity to reach 2.4 GHz. Fill
            # the preamble-to-first-tile window (~7.5 us to ~12 us) with
            # zero matmuls so the real stream starts at full clock.
            warm = wpool.tile([128, 256], F16, name="warm")
            nc.gpsimd.memset(warm[:], 0.0)
            warmps = psum.tile([128, 256], F32, name="warmps", tag="pt")
            for _ in range(20):
                nc.tensor.matmul(
                    warmps[:], warm[:, :128], warm[:], start=True, stop=True
                )
            # weights for the first WSPLIT batches: needed soon, small
            nc.gpsimd.dma_start(wt[:, 0:WSPLIT], w_d[:, 0:WSPLIT])

            xts = []
            for b in range(BPC):
                xt = xpool.tile([128, KC, T], F16, tag="xt")
                xts.append(xt)
                xsrc = x_d[b].rearrange("(k p) t -> p k t", p=128)
                if b == 0:
                    # 4 t-chunks so the first matmuls start ~2 us earlier
                    for tch in range(NC_):
                        nc.sync.dma_start(
                            xt[:, :, tch * NT:(tch + 1) * NT],
                            xsrc[:, :, tch * NT:(tch + 1) * NT],
                        )
                else:
                    nc.sync.dma_start(xt[:], xsrc)

            # Bulk weights: dispatch only after xt[1] has landed (tiny gpsimd
            # read of xt[1] creates the dependency), so the early HBM
            # bandwidth goes to x batches 0-2 instead.
            wgate = wpool.tile([128, 2], F16, name="wgate")
            nc.gpsimd.tensor_copy(wgate[:], xts[1][:, 0, 0:2])
            nc.gpsimd.dma_start(wt[:, WSPLIT:], w_d[:, WSPLIT:])

            for b in range(BPC):
                xt = xts[b]
                for m in range(MC):
                    # ot[p, t] fp16 (512 KiB, stored as soon as this m is done)
                    ot = opool.tile([128, T], F16, tag="ot")
                    for n in range(NC_):
                        pt = psum.tile([128, NT], F32)
                        for k in range(KC):
                            nc.tensor.matmul(
                                pt[:],
                                wt[:, b, k, m * 128:(m + 1) * 128],
                                xt[:, k, n * NT:(n + 1) * NT],
                                start=(k == 0),
                                stop=(k == KC - 1),
                            )
                        if n % 2 == 0:
                            nc.vector.tensor_copy(
                                ot[:, n * NT:(n + 1) * NT], pt[:]
                            )
                        else:
                            nc.scalar.copy(ot[:, n * NT:(n + 1) * NT], pt[:])
                        if b >= BPC - 2:
                            # tail: store each n-chunk as soon as it's cast
                            nc.gpsimd.dma_start(
                                o_d[b, m * 128:(m + 1) * 128, n * NT:(n + 1) * NT],
                                ot[:, n * NT:(n + 1) * NT],
                            )
                    if b < BPC - 2:
                        nc.gpsimd.dma_start(
                            o_d[b, m * 128:(m + 1) * 128, :], ot[:]
                        )

    nc.compile()
    return nc


def _get_compiled():
    global _compiled
    if _compiled is None:
        _compiled = _build()
    return _compiled


def _run(x, subjects, weights, **spmd_kwargs):
    x = np.asarray(x, dtype=np.float32)
    subjects = np.asarray(subjects).astype(np.int64)
    weights = np.asarray(weights, dtype=np.float32)

    x1 = x.astype(np.float16)              # (B, C, T) fp16
    w_g = weights[subjects].astype(np.float16)  # (B, C, D) fp16
    # wp[core][p, b, k, d] = w_g[core*BPC + b, k*128 + p, d]
    wp = np.ascontiguousarray(
        w_g.reshape(N_CORES, BPC, KC, 128, D).transpose(0, 3, 1, 2, 4)
    )

    nc = _get_compiled()
    in_maps = [
        {
            "x1": x1[i * BPC:(i + 1) * BPC],
            "wp": wp[i],
        }
        for i in range(N_CORES)
    ]
    res = run_bass_kernel_spmd(
        nc, in_maps, core_ids=list(range(N_CORES)), **spmd_kwargs
    )
    out = np.concatenate([r["out"] for r in res.results], axis=0).astype(
        np.float32
    )
    return out, res


def kernel(x, subjects, weights):
    return _run(x, subjects, weights)[0]


# revision 6
# speedup vs baseline: 1.2579x; 1.2579x over previous
"""Subject-routed batched matmul for Trainium2 (8 NeuronCores, SPMD data-parallel).

out[b, d, t] = sum_c x[b, c, t] * weights[subjects[b], c, d]

Strategy:
- Data-parallel over batch B=128 across 8 cores (16 batches each).
- Host-side: gather per-batch weights (weights[subjects], tiny), cast x and w
  to fp16. The tolerance gate is rel_err < 2e-2 and fp16-in/fp16-out measures
  ~5e-4, so single fp16 (2 B/elem) halves HBM traffic vs fp32: 34 MiB/core
  total -> ~100 us roofline at the ~358 GB/s per-core HBM limit.
- Device: per batch, out[b] (256d, 2048t) = w[b].T @ x[b], tiled K=2x128
  (contraction over c), M=2x128 (d -> PSUM partitions), N=4x512 (t, one
  PSUM bank per tile). 2 matmuls per PSUM bank, fp16 at 1 cycle/row.
- DMA model (from v1-v3 traces): each DMA ring serializes transfers with
  ~1.3-2 us of completion dead time between them, so per-queue throughput is
  duty-cycle-bound. Fixes here: bigger transfers (x in 2 MiB batch-pairs,
  output in 1 MiB whole-batch stores) and two store queues (even batches ->
  scalar/ACT HWDGE ring, odd batches -> gpsimd/SWDGE).
- PSUM f32 -> SBUF fp16 casts split: n even -> vector, n odd -> scalar
  (all-on-DVE was ~80 us busy in v1, too close to the critical path).
- PE warmup matmuls bridge the preamble so the HAM clock gate reaches
  2.4 GHz before the real stream starts.
"""

import sys

for _p in ("/opt/trn_rl_repo", "/root/.axon_site/_ro/trn_rl_repo"):
    if _p not in sys.path:
        sys.path.append(_p)

import numpy as np

import concourse.mybir as mybir
import concourse.tile as tile
from concourse import bacc
from concourse.bass_utils import run_bass_kernel_spmd

B, C, D, T, N_SUBJECTS = 128, 256, 256, 2048, 8
N_CORES = 8
BPC = B // N_CORES  # batches per core

KC = C // 128  # k chunks (contraction dim on partitions)
MC = D // 128  # m chunks (output partition dim)
NT = 512       # n tile (one PSUM bank of f32)
NC_ = T // NT  # n chunks

F32 = mybir.dt.float32
F16 = mybir.dt.float16

_compiled = None


def _build():
    nc = bacc.Bacc("TRN2", target_bir_lowering=False, debug=False)
    # x1[b, c, t] fp16 (plain reshape of the fp32 input, cast on host)
    # wp[p, b, k, d] fp16 — host-pre-packed to the SBUF layout so the weight
    # DMA is contiguous per partition.
    x_d = nc.dram_tensor("x1", [BPC, C, T], F16, kind="ExternalInput")
    w_d = nc.dram_tensor("wp", [128, BPC, KC, D], F16, kind="ExternalInput")
    o_d = nc.dram_tensor("out", [BPC, D, T], F16, kind="ExternalOutput")

    with tile.TileContext(nc) as tc:
        with (
            tc.tile_pool(name="wpool", bufs=1) as wpool,
            tc.tile_pool(name="xpool", bufs=4) as xpool,
            tc.tile_pool(name="opool", bufs=4) as opool,
            tc.tile_pool(name="psum", bufs=8, space="PSUM") as psum,
        ):
            wt = wpool.tile([128, BPC, KC, D], F16)
            # PE warmup: the HAM clock gate boots at 1.2 GHz and needs
            # ~3.4 us of sustained matmul activity to reach 2.4 GHz. Fill
            # the preamble-to-first-tile window (~7.5 to ~11 us) with zero
            # matmuls so the real stream starts at full clock.
            warm = wpool.tile([128, 256], F16, name="warm")
            nc.gpsimd.memset(warm[:], 0.0)
            warmps = psum.tile([128, 256], F32, name="warmps", tag="pt")
            for _ in range(16):
                nc.tensor.matmul(
                    warmps[:], warm[:, :128], warm[:], start=True, stop=True
                )
            # b=0's weights first (128 KiB -> lands ~10.5 us, with the first
            # x chunk), then the rest; both on the gpsimd SWDGE queue which
            # round-robins with the sync x stream at the SDMA engines.
            nc.gpsimd.dma_start(wt[:, 0:1], w_d[:, 0:1])
            nc.gpsimd.dma_start(wt[:, 1:], w_d[:, 1:])

            # x loads on sync: batch 0 in 2 chunks (fast start), batch 1
            # alone, then 2 MiB batch-pairs to amortize the per-transfer
            # completion dead time on the ring.
            xts = []  # xts[j] = tile holding batches 2j, 2j+1
            for j in range(BPC // 2):
                xt = xpool.tile([128, 2, KC, T], F16, tag="xt")
                xts.append(xt)
                if j == 0:
                    s0 = x_d[0].rearrange("(k p) t -> p k t", p=128)
                    half = T // 2
                    nc.sync.dma_start(xt[:, 0, :, :half], s0[:, :, :half])
                    nc.sync.dma_start(xt[:, 0, :, half:], s0[:, :, half:])
                    nc.sync.dma_start(
                        xt[:, 1], x_d[1].rearrange("(k p) t -> p k t", p=128)
                    )
                else:
                    nc.sync.dma_start(
                        xt[:],
                        x_d[2 * j:2 * j + 2].rearrange(
                            "b (k p) t -> p b k t", p=128
                        ),
                    )

            for b in range(BPC):
                xt = xts[b // 2]
                xb = b % 2
                # ot[p, m, t] fp16: whole batch (1 MiB), one store per batch
                ot = opool.tile([128, MC, T], F16, tag="ot")
                osink = nc.scalar if b % 2 == 0 else nc.gpsimd
                for m in range(MC):
                    for n in range(NC_):
                        pt = psum.tile([128, NT], F32)
                        for k in range(KC):
                            nc.tensor.matmul(
                                pt[:],
                                wt[:, b, k, m * 128:(m + 1) * 128],
                                xt[:, xb, k, n * NT:(n + 1) * NT],
                                start=(k == 0),
                                stop=(k == KC - 1),
                            )
                        if n % 2 == 0:
                            nc.vector.tensor_copy(
                                ot[:, m, n * NT:(n + 1) * NT], pt[:]
                            )
                        else:
                            nc.scalar.copy(
                                ot[:, m, n * NT:(n + 1) * NT], pt[:]
                            )
                    if b >= BPC - 2:
                        # tail: store per m-chunk as soon as it's cast
                        osink.dma_start(
                            o_d[b, m * 128:(m + 1) * 128, :], ot[:, m]
                        )
                if b < BPC - 2:
                    osink.dma_start(
                        o_d[b].rearrange("(m p) t -> p m t", p=128), ot[:]
                    )

    nc.compile()
    return nc


def _get_compiled():
    global _compiled
    if _compiled is None:
        _compiled = _build()
    return _compiled


def _run(x, subjects, weights, **spmd_kwargs):
    x = np.asarray(x, dtype=np.float32)
    subjects = np.asarray(subjects).astype(np.int64)
    weights = np.asarray(weights, dtype=np.float32)

    x1 = x.astype(np.float16)              # (B, C, T) fp16
    w_g = weights[subjects].astype(np.float16)  # (B, C, D) fp16
    # wp[core][p, b, k, d] = w_g[core*BPC + b, k*128 + p, d]
    wp = np.ascontiguousarray(
        w_g.reshape(N_CORES, BPC, KC, 128, D).transpose(0, 3, 1, 2, 4)
    )

    nc = _get_compiled()
    in_maps = [
        {
            "x1": x1[i * BPC:(i + 1) * BPC],
            "wp": wp[i],
        }
        for i in range(N_CORES)
    ]
    res = run_bass_kernel_spmd(
        nc, in_maps, core_ids=list(range(N_CORES)), **spmd_kwargs
    )
    out = np.concatenate([r["out"] for r in res.results], axis=0).astype(
        np.float32
    )
    return out, res


def kernel(x, subjects, weights):
    return _run(x, subjects, weights)[0]
